# revision 6
# baseline (speedup 1.0000x reference)
"""BSplineKAN layer forward on 8 TRN2 NeuronCores (Bass/Tile).

Math: the grids are uniform per input dim (knots t_j = t0 + j*h), so every
order-3 basis function is a shift of the cardinal cubic B-spline:
  B_c(x) = N3(u - c),  u = (x - t0)/h,  c = 0..10
  N3(v)  = [ (2-|v-2|)_+^3 - 4*(1-|v-2|)_+^3 ] / 6
N3 is bounded in [0, 2/3] (no cancellation blow-up), so the whole spline
contraction is a SINGLE fp16 matmul with K = 11*512:
  spline[b,o] = sum_{c,i} (coeffs[o,i,c]/6) * col_c(u[b,i]) ,
  col_c = (2-a)_+^3 - 4(1-a)_+^3 in [0,4],  a = |u - c - 2|
Then out = tanh(spline + silu(x) @ W.T + res_scale * x).

On-chip per column tile [128 i, 512 b]:
  a   = Abs(x*s + beta_c)            ACT (f32 internal, f16 out)
  t1m = (a sub 2) min 0 = -(2-a)_+   DVE tensor_scalar (fp16 4x mode)
  t2m = (a sub 1) min 0 = -(1-a)_+   DVE tensor_scalar
  s1 = t1m^2, s2 = t2m^2             Square: split ACT / DVE / Pool
  s2d = s2 * 4                       DVE tensor_scalar
  m1 = s1*t1m = -t1^3, m2 = s2d*t2m = -4 t2^3   tensor_tensor (DVE/Pool)
  col = m2 - m1 = t1^3 - 4 t2^3      tensor_tensor, fp16 out
(relative fp16 error ~5e-4 per step -> end-to-end rel err ~7e-3, gate 2e-2)

Data-parallel over batch across 8 cores; parameters replicated.
"""
import sys

sys.path.insert(0, "/opt/trn_rl_repo")

import numpy as np

from concourse import bacc
import concourse.bass as bass
import concourse.tile as tile
import concourse.mybir as mybir
from concourse.bass import ts
from concourse.bass_utils import run_bass_kernel_spmd
from concourse.masks import make_identity

DT = mybir.dt
AF = mybir.ActivationFunctionType
ALU = mybir.AluOpType

# problem shapes (hardcoded per contract)
B, I, O = 16384, 512, 512
NCOEF = 11                   # cardinal columns per input dim
N_CORES = 8
BC = B // N_CORES            # 2048 batch rows per core
CH = 512                     # batch chunk (free dim of compute tiles)
NCH = BC // CH               # 4 chunks
NIB = I // 128               # 4 input-dim blocks
NBI = CH // 128              # 4 batch sub-blocks per chunk

# engine-assignment tuning: of each column's two Squares, how they rotate
# across ACT(Square)/DVE(TT)/Pool(TT); and the mul/combine ops DVE vs Pool.
SQ_PATTERN = ("act", "dve", "act", "dve", "pool")   # rotation for squares
MUL_POOL_EVERY = 7                                  # every Nth mul -> Pool

_NC_CACHE = {}


def _build_nc(rep=1, has_res=False):
    """Build + compile the per-core Bass program (SPMD, identical on all cores).

    rep>1 wraps the whole computation in an on-device loop (for benchmarking:
    the work is repeated rep times so wall-clock slope isolates HW exec time).
    """
    nc = bacc.Bacc()
    x_d = nc.declare_dram_parameter("x", [BC, I], DT.float32, isOutput=False)
    d2_d = nc.declare_dram_parameter("d2", [NCOEF * I, O], DT.float16, isOutput=False)
    wt_d = nc.declare_dram_parameter("wt", [I, O], DT.float16, isOutput=False)
    ct_d = nc.declare_dram_parameter("ctab", [I, 16], DT.float32, isOutput=False)
    y_d = nc.declare_dram_parameter("y", [BC, O], DT.float32, isOutput=True)

    from contextlib import ExitStack
    with tile.TileContext(nc) as tc, ExitStack() as ctx:
        wp = ctx.enter_context(tc.tile_pool(name="weights", bufs=1))
        xap = ctx.enter_context(tc.tile_pool(name="xa", bufs=2))
        ap_ = ctx.enter_context(tc.tile_pool(name="a", bufs=2))
        tp1 = ctx.enter_context(tc.tile_pool(name="t1", bufs=2))
        tp2 = ctx.enter_context(tc.tile_pool(name="t2", bufs=2))
        sp1 = ctx.enter_context(tc.tile_pool(name="s1", bufs=2))
        sp2 = ctx.enter_context(tc.tile_pool(name="s2", bufs=2))
        mp1 = ctx.enter_context(tc.tile_pool(name="m1", bufs=2))
        mp2 = ctx.enter_context(tc.tile_pool(name="m2", bufs=2))
        rp = ctx.enter_context(tc.tile_pool(name="r", bufs=2))
        slp = ctx.enter_context(tc.tile_pool(name="sl", bufs=2))
        yop = ctx.enter_context(tc.tile_pool(name="yo", bufs=2))
        psx = ctx.enter_context(tc.tile_pool(name="psx", bufs=1, space="PSUM"))
        psa = ctx.enter_context(tc.tile_pool(name="psa", bufs=1, space="PSUM"))

        # ---- resident parameters ----
        d2sb = [[wp.tile([128, O], DT.float16, name=f"d2_{c}_{ib}", tag=f"d2_{c}_{ib}")
                 for ib in range(NIB)] for c in range(NCOEF)]
        for c in range(NCOEF):
            for ib in range(NIB):
                nc.sync.dma_start(out=d2sb[c][ib][:],
                                  in_=d2_d[(c * NIB + ib) * 128:(c * NIB + ib + 1) * 128, :])
        wsb = [wp.tile([128, O], DT.float16, name=f"w_{ib}", tag=f"w_{ib}")
               for ib in range(NIB)]
        for ib in range(NIB):
            nc.sync.dma_start(out=wsb[ib][:], in_=wt_d[ts(ib, 128), :])
        ct = [wp.tile([128, 16], DT.float32, name=f"ct_{ib}", tag=f"ct_{ib}")
              for ib in range(NIB)]
        for ib in range(NIB):
            nc.sync.dma_start(out=ct[ib][:], in_=ct_d[ts(ib, 128), :])
        ident = wp.tile([128, 128], DT.float32, name="ident", tag="ident")
        make_identity(nc, ident[:])

        n_groups = NCOEF * NIB + NIB   # accumulation groups per PSUM bank

        def _chunks():
            sqix = [0]
            mulix = [0]

            def square(dst, src):
                kind = SQ_PATTERN[sqix[0] % len(SQ_PATTERN)]
                sqix[0] += 1
                if kind == "act":
                    nc.scalar.activation(dst[:], src[:], AF.Square)
                elif kind == "dve":
                    nc.vector.tensor_mul(dst[:], src[:], src[:])
                else:
                    nc.gpsimd.tensor_mul(dst[:], src[:], src[:])

            def mul_eng():
                mulix[0] += 1
                return nc.gpsimd if mulix[0] % MUL_POOL_EVERY == 0 else nc.vector

            for ch in range(NCH):
                b0 = ch * CH
                # ---- load + transpose x chunk (xt lives in PSUM) ----
                xa = [xap.tile([128, I], DT.float32, name=f"xa{bi}", tag=f"xa{bi}")
                      for bi in range(NBI)]
                for bi in range(NBI):
                    nc.sync.dma_start(out=xa[bi][:],
                                      in_=x_d[b0 + bi * 128:b0 + (bi + 1) * 128, :])
                xt = [psx.tile([128, CH], DT.float32, name=f"xt{ib}", tag=f"xt{ib}")
                      for ib in range(NIB)]
                for ib in range(NIB):
                    for bi in range(NBI):
                        nc.tensor.transpose(xt[ib][:, ts(bi, 128)],
                                            xa[bi][:, ts(ib, 128)], ident[:])

                # ---- silu for base path (f16) ----
                sl = [slp.tile([128, CH], DT.float16, name=f"sl{ib}", tag=f"sl{ib}")
                      for ib in range(NIB)]
                for ib in range(NIB):
                    nc.scalar.activation(sl[ib][:], xt[ib][:], AF.Silu)

                acc = [psa.tile([128, O], DT.float32, name=f"acc{bs}", tag=f"acc{bs}")
                       for bs in range(NBI)]

                # ---- cardinal basis columns + matmuls ----
                gi = 0
                for c in range(NCOEF):
                    for ib in range(NIB):
                        a = ap_.tile([128, CH], DT.float16, name=f"a{ib}", tag=f"a{ib}")
                        nc.scalar.activation(a[:], xt[ib][:], AF.Abs,
                                             scale=ct[ib][:, 0:1],
                                             bias=ct[ib][:, 1 + c:2 + c])
                        t1m = tp1.tile([128, CH], DT.float16, name=f"t1{ib}", tag=f"t1{ib}")
                        nc.vector.tensor_scalar(t1m[:], a[:], 2.0, 0.0,
                                                ALU.subtract, ALU.min)
                        t2m = tp2.tile([128, CH], DT.float16, name=f"t2{ib}", tag=f"t2{ib}")
                        nc.vector.tensor_scalar(t2m[:], a[:], 1.0, 0.0,
                                                ALU.subtract, ALU.min)
                        s1 = sp1.tile([128, CH], DT.float16, name=f"s1{ib}", tag=f"s1{ib}")
                        square(s1, t1m)
                        s2 = sp2.tile([128, CH], DT.float16, name=f"s2{ib}", tag=f"s2{ib}")
                        square(s2, t2m)
                        s2d = sp2.tile([128, CH], DT.float16, name=f"s2d{ib}", tag=f"s2d{ib}")
                        nc.vector.tensor_scalar_mul(s2d[:], s2[:], 4.0)
                        m1 = mp1.tile([128, CH], DT.float16, name=f"m1{ib}", tag=f"m1{ib}")
                        mul_eng().tensor_mul(m1[:], s1[:], t1m[:])      # -t1^3
                        m2 = mp2.tile([128, CH], DT.float16, name=f"m2{ib}", tag=f"m2{ib}")
                        mul_eng().tensor_mul(m2[:], s2d[:], t2m[:])     # -4 t2^3
                        r = rp.tile([128, CH], DT.float16, name=f"r{ib}", tag=f"r{ib}")
                        mul_eng().tensor_sub(r[:], m2[:], m1[:])
                        for bs in range(NBI):
                            nc.tensor.matmul(acc[bs][:], r[:, ts(bs, 128)],
                                             d2sb[c][ib][:],
                                             start=(gi == 0),
                                             stop=(gi == n_groups - 1))
                        gi += 1

                # ---- base path matmuls (f16) ----
                for ib in range(NIB):
                    for bs in range(NBI):
                        nc.tensor.matmul(acc[bs][:], sl[ib][:, ts(bs, 128)],
                                         wsb[ib][:],
                                         start=(gi == 0),
                                         stop=(gi == n_groups - 1))
                    gi += 1

                # ---- epilogue: (+ res_scale*x), tanh, store ----
                for bs in range(NBI):
                    if has_res:
                        nc.vector.scalar_tensor_tensor(
                            acc[bs][:], xa[bs][:], ct[0][:, 15:16], acc[bs][:],
                            op0=ALU.mult, op1=ALU.add)
                    ot = yop.tile([128, O], DT.float32, name=f"ot{bs % 2}",
                                  tag=f"ot{bs % 2}")
                    nc.scalar.activation(ot[:], acc[bs][:], AF.Tanh)
                    nc.sync.dma_start(out=y_d[b0 + bs * 128:b0 + (bs + 1) * 128, :],
                                      in_=ot[:])

        if rep > 1:
            with tc.For_i(0, rep, 1):
                _chunks()
        else:
            _chunks()

    nc.compile()
    return nc


def _host_tables(coeffs, grid_steps_log, grid_start, base_weight, res_scale):
    """Precompute D2 weights + per-dim scale/bias table on the host."""
    steps = np.log1p(np.exp(grid_steps_log.astype(np.float64)))
    t0 = grid_start.astype(np.float64)[:, 0]
    h = steps.mean(axis=1)

    # matmul weights: coeffs/6 (the /6 of N3 folded here), layout (c, i, o)
    d2_dev = np.ascontiguousarray(
        (coeffs.astype(np.float64) / 6.0).transpose(2, 1, 0)
        .reshape(NCOEF * I, O).astype(np.float16))

    ctab = np.zeros((I, 16), dtype=np.float64)
    ctab[:, 0] = 1.0 / h                       # scale for a = |u - c - 2|
    for c in range(NCOEF):
        ctab[:, 1 + c] = -t0 / h - (c + 2.0)   # bias per column
    ctab[:, 15] = float(np.asarray(res_scale).reshape(-1)[0])
    ctab = ctab.astype(np.float32)

    wt = np.ascontiguousarray(base_weight.T.astype(np.float16))  # (I, O)
    return d2_dev, ctab, wt


def _get_nc(rep=1, has_res=False):
    key = (rep, has_res)
    if key not in _NC_CACHE:
        _NC_CACHE[key] = _build_nc(rep, has_res)
    return _NC_CACHE[key]


def host_input_map(x, d2_dev, ctab, wt):
    """Per-input list of per-core arrays (for external runners)."""
    return {
        "x": [np.ascontiguousarray(x[c * BC:(c + 1) * BC]) for c in range(N_CORES)],
        "d2": [d2_dev] * N_CORES,
        "wt": [wt] * N_CORES,
        "ctab": [ctab] * N_CORES,
    }


def run_on_device(x, d2_dev, ctab, wt, rep=1, trace=False, **kw):
    has_res = bool(ctab[0, 15] != 0.0)
    nc = _get_nc(rep, has_res)
    in_maps = []
    for c in range(N_CORES):
        in_maps.append({
            "x": np.ascontiguousarray(x[c * BC:(c + 1) * BC]),
            "d2": d2_dev, "wt": wt, "ctab": ctab,
        })
    res = run_bass_kernel_spmd(nc, in_maps, list(range(N_CORES)), trace=trace, **kw)
    y = np.concatenate([res.results[c]["y"] for c in range(N_CORES)], axis=0)
    return y, res


def kernel(x, coeffs, base_weight, grid_steps_log, grid_start, res_scale):
    x = np.asarray(x, dtype=np.float32)
    d2_dev, ctab, wt = _host_tables(
        np.asarray(coeffs), np.asarray(grid_steps_log), np.asarray(grid_start),
        np.asarray(base_weight), np.asarray(res_scale))
    y, _ = run_on_device(x, d2_dev, ctab, wt)
    return y


# revision 10
# speedup vs baseline: 1.9681x; 1.9681x over previous
"""BSplineKAN layer forward on 8 TRN2 NeuronCores (Bass/Tile).

Math: the grids are uniform per input dim (knots t_j = t0 + j*h), so every
order-3 basis function is a shift of the cardinal cubic B-spline:
  B_c(x) = N3(u - c),  u = (x - t0)/h,  c = 0..10
  N3(v)  = [ (2-a)_+^3 - 4*(1-a)_+^3 ] / 6,   a = |v - 2|
N3 is bounded (no cancellation blow-up), so the whole spline contraction
is a SINGLE fp16 matmul with K = 11*512:
  spline[b,o] = sum_{c,i} (coeffs[o,i,c]/6) * col_c(u[b,i])
  col_c = (2-a)_+^3 - 4(1-a)_+^3 in [0,4],  a = |u - c - 2|
Then out = tanh(spline + silu(x) @ W.T + res_scale * x).

col_c is produced from a = Abs(x*s + beta_c) (ACT engine) by one of two
engine-balanced strategies:
  - custom-DVE pair (2 fused Vector ops, registered at import):
      A = +t1^3          [sub,min,sq,neg,mul]
      B = A - 4*t2^3     [sub,min,sq,mul,*(-4),sub]  (B reads A via Src1)
  - ACT chain: t1=Relu(2-a), t2'=Relu(cbrt4*(1-a)), squares on ACT,
    cubes+combine as tensor_tensor ops on DVE/Pool.
The mix ratio balances ACT vs DVE vs Pool occupancy under the PE roof.

Data-parallel over batch across 8 cores; parameters replicated.
"""
import sys

sys.path.insert(0, "/opt/trn_rl_repo")

import numpy as np

import concourse.dve_ops as _dve_ops
from concourse.dve_ops import DveOp, OPS as _DVE_OPS, _SUB_OPCODE_FOR_NAME, \
    CUSTOM_DVE_SPECS
from concourse.dve_spec import Spec, Src0, Src1, C0, C1, C2, Zero, minn, sq, \
    lower as _dve_lower, _has_src1
from concourse.dve_uop import DveOpSpec

from concourse import bacc
import concourse.bass as bass
import concourse.tile as tile
import concourse.mybir as mybir
from concourse.bass import ts
from concourse.bass_utils import run_bass_kernel_spmd
from concourse.masks import make_identity

DT = mybir.dt
AF = mybir.ActivationFunctionType
ALU = mybir.AluOpType

# problem shapes (hardcoded per contract)
B, I, O = 16384, 512, 512
NCOEF = 11                   # cardinal columns per input dim
N_CORES = 8
BC = B // N_CORES            # 2048 batch rows per core
CH = 512                     # batch chunk (free dim of compute tiles)
NCH = BC // CH               # 4 chunks
NIB = I // 128               # 4 input-dim blocks
NBI = CH // 128              # 4 batch sub-blocks per chunk

CBRT4 = 4.0 ** (1.0 / 3.0)
# every Nth (c,ib) column-instance uses the ACT-heavy strategy (engine balance)
ACT_STRAT_EVERY = 8
# of the ACT-strategy cube muls, every Nth mul goes to DVE instead of Pool
_NC_CACHE = {}


def _ref_cube_neg(in0, in1, s0, s1, imm2):
    m = np.minimum(in0 - s0, 0.0)
    return -((m * m) * m)


def _ref_cube_acc(in0, in1, s0, s1, imm2):
    m = np.minimum(in0 - s0, 0.0)
    return in1 - ((m * m) * m) * imm2


def _register_ops():
    """Register the two fused column ops (idempotent)."""
    if "BSPL_CUBE_NEG_ANT" in _SUB_OPCODE_FOR_NAME:
        a = next(op for op in _DVE_OPS if op.name == "BSPL_CUBE_NEG_ANT")
        b = next(op for op in _DVE_OPS if op.name == "BSPL_CUBE_ACC_ANT")
        return a, b

    def make(name, spec):
        row = max(_SUB_OPCODE_FOR_NAME.values()) + 1
        assert row < 0x20
        _SUB_OPCODE_FOR_NAME[name] = row
        shas = {}
        for ver in ("v3", "v4"):
            uops = _dve_lower(spec, ver=ver)
            tmp = DveOpSpec(name=name, opcode=row, uops=uops,
                            rd1_en=_has_src1(spec))
            shas[ver] = tmp.sha(ver)
        op = DveOp(name, spec, subdim=False, uops_sha=shas)
        _DVE_OPS.append(op)
        CUSTOM_DVE_SPECS[name] = spec
        return op

    m1 = minn(Src0 - C0, Zero)
    spec_a = Spec(body=sq(m1) * (Zero - m1), reference=_ref_cube_neg)
    m2 = minn(Src0 - C0, Zero)
    spec_b = Spec(body=Src1 - sq(m2) * m2 * C2, reference=_ref_cube_acc)
    op_a = make("BSPL_CUBE_NEG_ANT", spec_a)   # out = +((s0-in0)_+)^3
    op_b = make("BSPL_CUBE_ACC_ANT", spec_b)   # out = in1 - imm2*(in0-s0|min0)^3
    return op_a, op_b


OP_CUBE, OP_CUBE_ACC = _register_ops()


def _build_nc(rep=1, has_res=False):
    """Build + compile the per-core Bass program (SPMD, identical on all cores).

    rep>1 wraps the whole computation in an on-device loop (for benchmarking:
    the work is repeated rep times so wall-clock slope isolates HW exec time).
    """
    nc = bacc.Bacc()
    x_d = nc.declare_dram_parameter("x", [BC, I], DT.float32, isOutput=False)
    d2_d = nc.declare_dram_parameter("d2", [NCOEF * I, O], DT.float16, isOutput=False)
    wt_d = nc.declare_dram_parameter("wt", [I, O], DT.float16, isOutput=False)
    ct_d = nc.declare_dram_parameter("ctab", [I, 16], DT.float32, isOutput=False)
    y_d = nc.declare_dram_parameter("y", [BC, O], DT.float32, isOutput=True)

    from contextlib import ExitStack
    with tile.TileContext(nc) as tc, ExitStack() as ctx:
        wp = ctx.enter_context(tc.tile_pool(name="weights", bufs=1))
        xap = ctx.enter_context(tc.tile_pool(name="xa", bufs=2))
        ap_ = ctx.enter_context(tc.tile_pool(name="a", bufs=3))
        aap = ctx.enter_context(tc.tile_pool(name="aa", bufs=3))
        sap = ctx.enter_context(tc.tile_pool(name="sa", bufs=2))
        rp = ctx.enter_context(tc.tile_pool(name="r", bufs=3))
        slp = ctx.enter_context(tc.tile_pool(name="sl", bufs=2))
        yop = ctx.enter_context(tc.tile_pool(name="yo", bufs=2))
        psx = ctx.enter_context(tc.tile_pool(name="psx", bufs=1, space="PSUM"))
        psa = ctx.enter_context(tc.tile_pool(name="psa", bufs=1, space="PSUM"))

        # ---- resident parameters ----
        d2sb = [[wp.tile([128, O], DT.float16, name=f"d2_{c}_{ib}", tag=f"d2_{c}_{ib}")
                 for ib in range(NIB)] for c in range(NCOEF)]
        for c in range(NCOEF):
            for ib in range(NIB):
                nc.sync.dma_start(out=d2sb[c][ib][:],
                                  in_=d2_d[(c * NIB + ib) * 128:(c * NIB + ib + 1) * 128, :])
        wsb = [wp.tile([128, O], DT.float16, name=f"w_{ib}", tag=f"w_{ib}")
               for ib in range(NIB)]
        for ib in range(NIB):
            nc.sync.dma_start(out=wsb[ib][:], in_=wt_d[ts(ib, 128), :])
        ct = [wp.tile([128, 16], DT.float32, name=f"ct_{ib}", tag=f"ct_{ib}")
              for ib in range(NIB)]
        for ib in range(NIB):
            nc.sync.dma_start(out=ct[ib][:], in_=ct_d[ts(ib, 128), :])
        ident = wp.tile([128, 128], DT.float32, name="ident", tag="ident")
        make_identity(nc, ident[:])

        n_groups = NCOEF * NIB + NIB   # accumulation groups per PSUM bank

        def _chunks():
            colix = [0]
            mulix = [0]

            for ch in range(NCH):
                b0 = ch * CH
                # ---- load + transpose x chunk (xt lives in PSUM) ----
                xa = [xap.tile([128, I], DT.float32, name=f"xa{bi}", tag=f"xa{bi}")
                      for bi in range(NBI)]
                for bi in range(NBI):
                    nc.sync.dma_start(out=xa[bi][:],
                                      in_=x_d[b0 + bi * 128:b0 + (bi + 1) * 128, :])
                xt = [psx.tile([128, CH], DT.float32, name=f"xt{ib}", tag=f"xt{ib}")
                      for ib in range(NIB)]
                for ib in range(NIB):
                    for bi in range(NBI):
                        nc.tensor.transpose(xt[ib][:, ts(bi, 128)],
                                            xa[bi][:, ts(ib, 128)], ident[:])

                # ---- silu for base path (f16) ----
                sl = [slp.tile([128, CH], DT.float16, name=f"sl{ib}", tag=f"sl{ib}")
                      for ib in range(NIB)]
                for ib in range(NIB):
                    nc.scalar.activation(sl[ib][:], xt[ib][:], AF.Silu)

                acc = [psa.tile([128, O], DT.float32, name=f"acc{bs}", tag=f"acc{bs}")
                       for bs in range(NBI)]

                # ---- basis columns + matmuls ----
                gi = 0
                for c in range(NCOEF):
                    # phase 1: a-tiles for all ibs (ACT), interleaves with
                    # phase-2 DVE work of the previous c
                    a_t = []
                    strat = []
                    for ib in range(NIB):
                        use_act = (colix[0] % ACT_STRAT_EVERY) == (ACT_STRAT_EVERY - 1)
                        colix[0] += 1
                        strat.append(use_act)
                        a = ap_.tile([128, CH], DT.float16, name=f"a{ib}", tag=f"a{ib}")
                        nc.scalar.activation(a[:], xt[ib][:], AF.Abs,
                                             scale=ct[ib][:, 0:1],
                                             bias=ct[ib][:, 1 + c:2 + c])
                        a_t.append(a)
                    # phase 2
                    r_t = []
                    if not all(strat):
                        # custom A ops first (pairs interleave in the DVE queue)
                        A_t = {}
                        for ib in range(NIB):
                            if strat[ib]:
                                continue
                            A = aap.tile([128, CH], DT.float16, name=f"A{ib}", tag=f"A{ib}")
                            nc.vector._custom_dve(OP_CUBE, out=A[:], in0=a_t[ib][:],
                                                  s0=2.0)
                            A_t[ib] = A
                    for ib in range(NIB):
                        r = rp.tile([128, CH], DT.float16, name=f"r{ib}", tag=f"r{ib}")
                        if not strat[ib]:
                            nc.vector._custom_dve(OP_CUBE_ACC, out=r[:], in0=a_t[ib][:],
                                                  in1=A_t[ib][:], s0=1.0, imm2=-4.0)
                        else:
                            # ACT-heavy strategy
                            t1 = sap.tile([128, CH], DT.float16, name=f"t1{ib}", tag=f"t1{ib}")
                            nc.scalar.activation(t1[:], a_t[ib][:], AF.Relu,
                                                 scale=-1.0, bias=ct[ib][:, 12:13])
                            t2 = sap.tile([128, CH], DT.float16, name=f"t2{ib}", tag=f"t2{ib}")
                            nc.scalar.activation(t2[:], a_t[ib][:], AF.Relu,
                                                 scale=-CBRT4, bias=ct[ib][:, 13:14])
                            s1 = sap.tile([128, CH], DT.float16, name=f"s1{ib}", tag=f"s1{ib}")
                            nc.scalar.activation(s1[:], t1[:], AF.Square)
                            s2 = sap.tile([128, CH], DT.float16, name=f"s2{ib}", tag=f"s2{ib}")
                            nc.scalar.activation(s2[:], t2[:], AF.Square)
                            m1 = sap.tile([128, CH], DT.float16, name=f"m1{ib}", tag=f"m1{ib}")
                            m2 = sap.tile([128, CH], DT.float16, name=f"m2{ib}", tag=f"m2{ib}")
                            # cube muls + combine on Pool (occasionally DVE)
                            e1 = nc.gpsimd if mulix[0] % 3 else nc.vector
                            mulix[0] += 1
                            e2 = nc.gpsimd if mulix[0] % 3 else nc.vector
                            mulix[0] += 1
                            e1.tensor_mul(m1[:], s1[:], t1[:])   # t1^3
                            e2.tensor_mul(m2[:], s2[:], t2[:])   # 4*t2^3
                            e3 = nc.gpsimd if mulix[0] % 3 else nc.vector
                            mulix[0] += 1
                            e3.tensor_sub(r[:], m1[:], m2[:])
                        r_t.append(r)
                        for bs in range(NBI):
                            nc.tensor.matmul(acc[bs][:], r[:, ts(bs, 128)],
                                             d2sb[c][ib][:],
                                             start=(gi == 0),
                                             stop=(gi == n_groups - 1))
                        gi += 1

                # ---- base path matmuls (f16) ----
                for ib in range(NIB):
                    for bs in range(NBI):
                        nc.tensor.matmul(acc[bs][:], sl[ib][:, ts(bs, 128)],
                                         wsb[ib][:],
                                         start=(gi == 0),
                                         stop=(gi == n_groups - 1))
                    gi += 1

                # ---- epilogue: (+ res_scale*x), tanh, store ----
                for bs in range(NBI):
                    if has_res:
                        nc.vector.scalar_tensor_tensor(
                            acc[bs][:], xa[bs][:], ct[0][:, 15:16], acc[bs][:],
                            op0=ALU.mult, op1=ALU.add)
                    ot = yop.tile([128, O], DT.float32, name=f"ot{bs % 2}",
                                  tag=f"ot{bs % 2}")
                    nc.scalar.activation(ot[:], acc[bs][:], AF.Tanh)
                    nc.sync.dma_start(out=y_d[b0 + bs * 128:b0 + (bs + 1) * 128, :],
                                      in_=ot[:])

        if rep > 1:
            with tc.For_i(0, rep, 1):
                _chunks()
        else:
            _chunks()

    nc.compile()
    return nc


def _host_tables(coeffs, grid_steps_log, grid_start, base_weight, res_scale):
    """Precompute D2 weights + per-dim scale/bias table on the host."""
    steps = np.log1p(np.exp(grid_steps_log.astype(np.float64)))
    t0 = grid_start.astype(np.float64)[:, 0]
    h = steps.mean(axis=1)

    # matmul weights: coeffs/6 (the /6 of N3 folded here), layout (c, i, o)
    d2_dev = np.ascontiguousarray(
        (coeffs.astype(np.float64) / 6.0).transpose(2, 1, 0)
        .reshape(NCOEF * I, O).astype(np.float16))

    ctab = np.zeros((I, 16), dtype=np.float64)
    ctab[:, 0] = 1.0 / h                       # scale for a = |u - c - 2|
    for c in range(NCOEF):
        ctab[:, 1 + c] = -t0 / h - (c + 2.0)   # bias per column
    ctab[:, 12] = 2.0                          # ACT-strategy t1 bias
    ctab[:, 13] = CBRT4                        # ACT-strategy t2 bias
    ctab[:, 15] = float(np.asarray(res_scale).reshape(-1)[0])
    ctab = ctab.astype(np.float32)

    wt = np.ascontiguousarray(base_weight.T.astype(np.float16))  # (I, O)
    return d2_dev, ctab, wt


def _get_nc(rep=1, has_res=False):
    key = (rep, has_res)
    if key not in _NC_CACHE:
        _NC_CACHE[key] = _build_nc(rep, has_res)
    return _NC_CACHE[key]


def host_input_map(x, d2_dev, ctab, wt):
    """Per-input list of per-core arrays (for external runners)."""
    return {
        "x": [np.ascontiguousarray(x[c * BC:(c + 1) * BC]) for c in range(N_CORES)],
        "d2": [d2_dev] * N_CORES,
        "wt": [wt] * N_CORES,
        "ctab": [ctab] * N_CORES,
    }


def run_on_device(x, d2_dev, ctab, wt, rep=1, trace=False, **kw):
    has_res = bool(ctab[0, 15] != 0.0)
    nc = _get_nc(rep, has_res)
    in_maps = []
    for c in range(N_CORES):
        in_maps.append({
            "x": np.ascontiguousarray(x[c * BC:(c + 1) * BC]),
            "d2": d2_dev, "wt": wt, "ctab": ctab,
        })
    res = run_bass_kernel_spmd(nc, in_maps, list(range(N_CORES)), trace=trace, **kw)
    y = np.concatenate([res.results[c]["y"] for c in range(N_CORES)], axis=0)
    return y, res


def kernel(x, coeffs, base_weight, grid_steps_log, grid_start, res_scale):
    x = np.asarray(x, dtype=np.float32)
    d2_dev, ctab, wt = _host_tables(
        np.asarray(coeffs), np.asarray(grid_steps_log), np.asarray(grid_start),
        np.asarray(base_weight), np.asarray(res_scale))
    y, _ = run_on_device(x, d2_dev, ctab, wt)
    return y
